# revision 45
# baseline (speedup 1.0000x reference)
"""TransformerXL relative attention on 8 TRN2 NeuronCores.

Sharding: 16 heads -> 2 heads per core (tensor parallel). Each core computes
its column shard of the Q/K/V/R projections, full-batch attention for its two
heads, and the full-batch row-sharded output projection, producing a partial
[B*Q, D] output. The host sums the 8 partials (row-parallel matmul => the
all-reduce is a host-side sum).

v16 = v3 dataflow + fine-grained phase fusion:
  - The attention phase for batch b is EMISSION-INTERLEAVED with the
    projection/position phase of batch b+2 (and with the previous batch's
    output projection at the tail): after each content-matmul pair the
    emitter drops in one projection-chain unit (8 back-to-back matmuls).
    The engine queues are FIFO, so this keeps ~2us of independent PE work
    between every dependent attention step -- the PE no longer drains
    while DVE adds / ACT exps catch up, which both hides the softmax
    latency chain and keeps the HAM clock gate at full rate (isolated
    attention phases ran at the 1.2 GHz cold clock).
  - PE warmup matmuls during the initial loads (HAM ramp); posC/wr lead
    the scalar ring; the first ref chunks ride the idle sync ring.
  - each shifted XBAR-transposed st read is split into 2 chunks so the
    first adds start at half the transpose latency (4-way split loses:
    per-transpose fixed cost dominates); the ybuf write rides the sync
    ring directly ahead of its st chunks (no gpsimd->sync sem hop).
  - replicated-denominator attn@V: the lhsT is [128,128] = [V_h | 64
    ones columns], so o_ps rows 64:128 hold the softmax denominator
    replicated across partitions and the 1/sum epilogue is two
    same-queue DVE ops (reciprocal + multiply) with ZERO DMA hops. The
    old DRAM-broadcast chain crossed 4 engine queues at ~8-12us latency
    per head and its tiny reciprocal queued behind bulk filler work;
    this trick only pays off inside the fused schedule (standalone it
    lost to the [64,512] reciprocal cost).
  - V^T for both heads comes from ONE full-partition [128,1024] XBAR
    transpose per batch (64-partition sources transpose at ~half rate).
  - content/position biases folded into the Q-projection PSUM drain via
    ACT per-partition bias APs (removes the rank-1 matmuls and shortens
    the q chain); with the epilogue now DVE-resident, the projection
    and output drains all run on ACT to balance (DVE ~122us busy was
    2nd after PE; ACT had 45us of slack).
  - filler distribution balanced across ALL attention phases: pp3's
    units split between at1 and at2, and the output-projection phases
    folded in as fillers (schedule: pp0 pp1 [at0|pp2] [at1|pp3a+y0]
    [at2|pp3b+y1] [at3|y2] y3), so the tail attention phases keep
    independent PE work in every fill slot instead of running dry.

Measured (8-core SPMD, one traced run each): v3 baseline 232-270us
across runs (median ~259); this version 198-209us (median ~201, best
198.6). rel err 1.29e-3.
"""

import numpy as np

import concourse.bass as bass
import concourse.mybir as mybir
import concourse.tile as tile
from concourse import bacc
from concourse.bass_utils import run_bass_kernel_spmd

B, Q, M, D, H = 4, 512, 512, 1024, 16
S = D // H          # 64
R = Q + M           # 1024
NCORES = 8
HPC = H // NCORES   # heads per core = 2
HS = HPC * S        # per-core head-channel width = 128
BR = B * R          # 4096
BQ = B * Q          # 2048

FP16 = mybir.dt.float16
FP32 = mybir.dt.float32
AF = mybir.ActivationFunctionType

_CACHED_NC = None


def build_nc():
    nc = bacc.Bacc()

    refC = nc.declare_dram_parameter("refC", [BR // 512, 128, 8 * 512],
                                     FP16, isOutput=False)
    posC = nc.declare_dram_parameter("posC", [R // 512, 128, 8 * 512],
                                     FP16, isOutput=False)
    wq = nc.declare_dram_parameter("wq", [128, 8 * 128], FP16,
                                   isOutput=False)
    wk = nc.declare_dram_parameter("wk", [128, 8 * 128], FP16,
                                   isOutput=False)
    wv = nc.declare_dram_parameter("wv", [128, 8 * 128], FP16,
                                   isOutput=False)
    wr = nc.declare_dram_parameter("wr", [128, 8 * 128], FP16,
                                   isOutput=False)
    wo = nc.declare_dram_parameter("wo", [HS, D], FP16, isOutput=False)
    cb = nc.declare_dram_parameter("cb", [HS, 1], FP32, isOutput=False)
    pb = nc.declare_dram_parameter("pb", [HS, 1], FP32, isOutput=False)
    y_out = nc.declare_dram_parameter("out", [BQ, D], FP16, isOutput=True)

    DT = D // 128  # 8 contraction tiles
    KT = R // 128  # 8 key tiles per batch row-block

    with tile.TileContext(nc) as tc:
        with (
            tc.tile_pool(name="consts", bufs=1) as consts,
            tc.tile_pool(name="inputs", bufs=1) as inputs,
            tc.tile_pool(name="acts", bufs=1) as acts,
            tc.tile_pool(name="work", bufs=1) as work,
            tc.tile_pool(name="ps", bufs=1, space="PSUM") as ps,
            tc.tile_pool(name="dram", bufs=1, space="DRAM") as dram,
        ):
            # PE warmup: keep the HAM activity window busy while the first
            # parameter DMAs land, so real matmuls start at full clock.
            warm = consts.tile([128, 512], FP16, tag="warm")
            nc.vector.memset(warm, 0.125)
            for _ in range(24):
                ps_w = ps.tile([128, 512], FP32, tag="proj", bufs=2)
                nc.tensor.matmul(ps_w, warm[:, 0:128], warm,
                                 start=True, stop=True)

            def load_w(param, name):
                t = consts.tile([128, DT, 128], FP16, tag=name)
                nc.scalar.dma_start(
                    out=t, in_=param.rearrange("p (dt m) -> p dt m", dt=DT)
                )
                return t

            pos_cs = []
            ref_cs = []

            def load_chunk(view, c, lst, nm, queue=None):
                rc = inputs.tile([128, DT, 512], FP16, tag="ref", bufs=4,
                                 name=f"{nm}{c}")
                q = queue or nc.scalar
                q.dma_start(
                    out=rc,
                    in_=view[c].rearrange("p (dt j) -> p dt j", dt=DT),
                )
                lst.append(rc)

            load_chunk(posC, 0, pos_cs, "pos")
            wr_sb = load_w(wr, "wr")
            load_chunk(posC, 1, pos_cs, "pos")
            wk_sb = load_w(wk, "wk")
            wq_sb = load_w(wq, "wq")
            wv_sb = load_w(wv, "wv")
            load_chunk(refC, 0, ref_cs, "ref", queue=nc.sync)
            load_chunk(refC, 1, ref_cs, "ref", queue=nc.sync)
            wo_sb = consts.tile([HS, D], FP16, tag="wo")
            nc.scalar.dma_start(out=wo_sb, in_=wo[:, :])
            cb_sb = consts.tile([HS, 1], FP32, tag="cb")
            nc.scalar.dma_start(out=cb_sb, in_=cb[:, :])
            pb_sb = consts.tile([HS, 1], FP32, tag="pb")
            nc.scalar.dma_start(out=pb_sb, in_=pb[:, :])

            # persistent activations (all fp16)
            k_sbs = []
            qcb_sbs = []
            qpb_sbs = []
            o_sbs = []
            vt_bs = {}
            for bb in range(B):
                k_sbs.append(acts.tile([HS, R], FP16, tag=f"k{bb}",
                                       name=f"k{bb}"))
                qcb_sbs.append(acts.tile([HS, 512], FP16, tag=f"qcb{bb}",
                                         name=f"qcb{bb}"))
                qpb_sbs.append(acts.tile([HS, 512], FP16, tag=f"qpb{bb}",
                                         name=f"qpb{bb}"))
                o_sbs.append(acts.tile([HS, 512], FP16, tag=f"o{bb}",
                                       name=f"o{bb}"))
            rel_sb = acts.tile([HS, R], FP16, tag="rel_sb")
            v_sb = acts.tile([HS, BR], FP16, tag="v_sb")
            recbuf = dram.tile([B, 1024], FP16, tag="recbuf")
            st_alls = {}

            # rel projection (needed by every position phase)
            for c in range(R // 512):
                ps_r = ps.tile([128, 512], FP32, tag="proj", bufs=2)
                for dt in range(DT):
                    nc.tensor.matmul(
                        ps_r, wr_sb[:, dt, :], pos_cs[c][:, dt, :],
                        start=(dt == 0), stop=(dt == DT - 1),
                    )
                nc.scalar.activation(
                    rel_sb[:, c * 512:(c + 1) * 512], ps_r, AF.Copy
                )

            def emit_proj(wt, c, dst):
                """One projection chain: dst <- wt.T @ ref chunk c."""
                ps_t = ps.tile([128, 512], FP32, tag="proj", bufs=2)
                for dt in range(DT):
                    nc.tensor.matmul(
                        ps_t, wt[:, dt, :], ref_cs[c][:, dt, :],
                        start=(dt == 0), stop=(dt == DT - 1),
                    )
                nc.scalar.activation(dst, ps_t, AF.Copy)

            def emit_q(b):
                c = 2 * b + 1
                ps_q = ps.tile([128, 512], FP32, tag="proj", bufs=2)
                for dt in range(DT):
                    nc.tensor.matmul(
                        ps_q, wq_sb[:, dt, :], ref_cs[c][:, dt, :],
                        start=(dt == 0), stop=(dt == DT - 1),
                    )
                nc.scalar.activation(qcb_sbs[b], ps_q, AF.Identity,
                                     bias=cb_sb)
                nc.scalar.activation(qpb_sbs[b], ps_q, AF.Identity,
                                     bias=pb_sb)

            def emit_pos_mms(b, p_alls, qts):
                """Position-logit matmul pairs + PSUM drains for query
                tiles `qts` of batch b (heads paired in the PE array)."""
                for qt in qts:
                    for kh in range(2):
                        pss = []
                        for h in range(HPC):
                            hsl = slice(h * S, (h + 1) * S)
                            ps_p = ps.tile([128, 512], FP32,
                                           tag="pos", bufs=2, name="ps_p")
                            nc.tensor.matmul(
                                ps_p,
                                qpb_sbs[b][hsl, qt * 128:(qt + 1) * 128],
                                rel_sb[hsl, kh * 512:(kh + 1) * 512],
                                start=True, stop=True,
                                tile_position=(h * S, 0),
                            )
                            pss.append(ps_p)
                        for h in range(HPC):
                            dst = p_alls[h][:, qt, 1 + kh * 512:
                                            1 + (kh + 1) * 512]
                            if h == 0:
                                nc.scalar.activation(dst, pss[h], AF.Copy)
                            else:
                                nc.vector.tensor_copy(dst, pss[h])

            def emit_pos_rt(b, p_alls, h):
                """DRAM round trip for head h: contiguous ybuf write then
                the shifted XBAR-transposed read back."""
                ybuf = dram.tile(
                    [Q, R + 1], FP16, tag=f"ybuf{b}_{h}",
                    name=f"ybuf{b}_{h}",
                )
                nc.sync.dma_start(
                    out=ybuf.rearrange("(qt p) c -> p qt c", p=128),
                    in_=p_alls[h],
                )
                shifted = (
                    ybuf.rearrange("a b -> (a b)")[Q: Q + Q * R]
                    .rearrange("(q r) -> q r", r=R)
                )
                st_all = work.tile(
                    [128, KT, 512], FP16,
                    tag="st", name=f"st{b}_{h}", bufs=5,
                )
                for cc in range(2):
                    nc.sync.dma_start(
                        out=st_all[:, cc * 4:(cc + 1) * 4, :],
                        in_=shifted[:, cc * 512:(cc + 1) * 512],
                        transpose=True,
                    )
                st_alls[(b, h)] = st_all

            def emit_vt(b):
                """V^T (ones-augmented) for batch b: ONE full-partition
                [128,1024] XBAR transpose covers both heads (a 64-partition
                source transposes at ~half efficiency), then per-head
                ones-augmented lhsT tiles are sliced out on the DVE."""
                vt_both = work.tile(
                    [128, KT, 128], FP16,
                    tag="vt_both", name=f"vt_both{b}", bufs=2,
                )
                nc.sync.dma_start(
                    out=vt_both,
                    in_=v_sb[:, b * R:(b + 1) * R],
                    transpose=True,
                )
                for h in range(HPC):
                    vt_all = acts.tile(
                        [128, KT, 128], FP16,
                        tag=f"vt{b}_{h}", name=f"vt{b}_{h}",
                    )
                    nc.vector.memset(vt_all[:, :, S:], 1.0)
                    nc.vector.tensor_copy(
                        vt_all[:, :, 0:S],
                        vt_both[:, :, h * S:(h + 1) * S],
                    )
                    vt_bs[(b, h)] = vt_all

            def projpos_units(b):
                """emit_projpos(b) as a list of closures, each ~one
                projection chain of PE work, to interleave with attention."""
                p_alls = []
                for h in range(HPC):
                    p_all = work.tile(
                        [128, Q // 128, R + 1], FP16,
                        tag="p_all", name=f"p_all{b}_{h}", bufs=4,
                    )
                    nc.vector.memset(p_all[:, :, 0:1], 0.0)
                    p_alls.append(p_all)

                def u_k0():
                    if b < B - 1:
                        load_chunk(refC, 2 * b + 2, ref_cs, "ref")
                        load_chunk(refC, 2 * b + 3, ref_cs, "ref")
                    emit_proj(wk_sb, 2 * b, k_sbs[b][:, 0:512])

                return [
                    u_k0,
                    lambda: emit_proj(wv_sb, 2 * b,
                                      v_sb[:, b * R:b * R + 512]),
                    lambda: emit_q(b),
                    lambda: emit_pos_mms(b, p_alls, [0, 1]),
                    lambda: emit_pos_mms(b, p_alls, [2, 3]),
                    lambda: emit_pos_rt(b, p_alls, 0),
                    lambda: emit_proj(wk_sb, 2 * b + 1,
                                      k_sbs[b][:, 512:1024]),
                    lambda: emit_pos_rt(b, p_alls, 1),
                    lambda: emit_proj(wv_sb, 2 * b + 1,
                                      v_sb[:, b * R + 512:b * R + R]),
                    lambda: emit_vt(b),
                ]

            def outproj_units(b):
                def mk(t):
                    def u():
                        y_sb = work.tile([128, D], FP16, tag="y_sb", bufs=4)
                        for j in range(2):
                            y_ps = ps.tile([128, 512], FP32, tag="proj",
                                           bufs=2, name="y_ps")
                            nc.tensor.matmul(
                                y_ps,
                                o_sbs[b][:, t * 128:(t + 1) * 128],
                                wo_sb[:, j * 512:(j + 1) * 512],
                                start=True, stop=True,
                            )
                            nc.scalar.activation(
                                y_sb[:, j * 512:(j + 1) * 512], y_ps,
                                AF.Copy,
                            )
                        nc.scalar.dma_start(
                            out=y_out[(b * 4 + t) * 128:
                                      (b * 4 + t + 1) * 128, :],
                            in_=y_sb,
                        )
                    return u
                return [mk(t) for t in range(4)]

            def emit_attn(b, fillers):
                """Attention for batch b, with one filler unit (a chunk of
                independent PE work) emitted after each dependent step so
                the PE queue never drains while adds/exps catch up."""
                fillers = list(fillers)

                def fill():
                    if fillers:
                        fillers.pop(0)()

                exs = {h: [] for h in range(HPC)}
                for K in range(KT):
                    cts = []
                    for h in range(HPC):
                        hsl = slice(h * S, (h + 1) * S)
                        ct = ps.tile([128, 512], FP32, tag="ct",
                                     bufs=2, name=f"ct{h}")
                        nc.tensor.matmul(
                            ct,
                            k_sbs[b][hsl, K * 128:(K + 1) * 128],
                            qcb_sbs[b][hsl, :],
                            start=True, stop=True,
                            tile_position=(h * S, 0),
                        )
                        cts.append(ct)
                    l_sb = work.tile([128, 2, 512], FP16, tag="l_sb",
                                     name="l_sb", bufs=6)
                    for h in range(HPC):
                        nc.vector.tensor_add(
                            l_sb[:, h, :], cts[h], st_alls[(b, h)][:, K, :]
                        )
                    ex = work.tile([128, 2, 512], FP16, tag="ex",
                                   name="ex", bufs=10)
                    nc.scalar.activation(
                        ex, l_sb, AF.Exp, scale=1.0 / np.sqrt(S)
                    )
                    for h in range(HPC):
                        exs[h].append(ex[:, h, :])
                    fill()
                for h in range(HPC):
                    hsl = slice(h * S, (h + 1) * S)
                    o_ps = ps.tile([128, 512], FP32, tag="ov",
                                   bufs=2, name=f"o_ps{h}")
                    for K in range(KT):
                        nc.tensor.matmul(
                            o_ps,
                            vt_bs[(b, h)][:, K, :],
                            exs[h][K],
                            start=(K == 0), stop=(K == KT - 1),
                        )
                    # o_ps rows 64:128 hold the softmax denominator
                    # replicated across 64 partitions (the ones columns of
                    # vt), so the 1/sum epilogue is two same-queue DVE ops
                    # with zero DMA hops (the old DRAM-broadcast chain
                    # crossed 4 queues at ~8-12us latency)
                    rec_h = work.tile([S, 512], FP16, tag="rec",
                                      name=f"rec{h}", bufs=2)
                    with nc.allow_low_precision(reason="softmax 1/sum fp16"):
                        nc.vector.reciprocal(rec_h, o_ps[S:, :])
                    nc.vector.tensor_mul(
                        o_sbs[b][hsl, :],
                        o_ps[0:S, :],
                        rec_h,
                    )
                    fill()
                while fillers:
                    fillers.pop(0)()

            def emit_projpos(b):
                for u in projpos_units(b):
                    u()

            # schedule: pp0 pp1 [at0|pp2] [at1|pp3a+y0] [at2|pp3b+y1]
            # [at3|y2] y3 -- pp3's units split across at1/at2 and the y
            # phases folded in as fillers, so the tail attention phases
            # keep independent PE work in every fill slot.
            emit_projpos(0)
            emit_projpos(1)
            emit_attn(0, projpos_units(2))
            pp3 = projpos_units(3)
            emit_attn(1, pp3[0:6] + outproj_units(0))
            emit_attn(2, pp3[6:10] + outproj_units(1))
            emit_attn(3, outproj_units(2))
            for u in outproj_units(3):
                u()

    nc.compile()
    return nc


def _make_in_maps(inputs):
    qs = np.asarray(inputs["query_seqs"], dtype=np.float32)
    pos = np.asarray(inputs["positional_encoding"], dtype=np.float32)
    mem = np.asarray(inputs["memory_seqs"], dtype=np.float32)
    wq = np.asarray(inputs["w_query"], dtype=np.float32)
    wk = np.asarray(inputs["w_key"], dtype=np.float32)
    wv = np.asarray(inputs["w_value"], dtype=np.float32)
    wr = np.asarray(inputs["w_r"], dtype=np.float32)
    wo = np.asarray(inputs["w_output"], dtype=np.float32)
    cb = np.asarray(inputs["content_bias"], dtype=np.float32)
    pb = np.asarray(inputs["position_bias"], dtype=np.float32)

    DT = D // 128

    def swz_w(w):
        # [D, HS] -> [128, DT*HS]: row p holds dt-major 128-blocks so the
        # SBUF load is per-partition contiguous.
        return np.ascontiguousarray(
            w.reshape(DT, 128, HS).transpose(1, 0, 2).reshape(128, DT * HS)
        ).astype(np.float16)

    def swz_x(xT, n):
        # [D, N] -> [N//512, 128, DT*512] chunk-major / partition / dt-major
        return np.ascontiguousarray(
            xT.reshape(DT, 128, n // 512, 512)
            .transpose(2, 1, 0, 3)
            .reshape(n // 512, 128, DT * 512)
        ).astype(np.float16)

    ref = np.concatenate([mem, qs], axis=1)  # [B, R, D]
    refT = np.ascontiguousarray(ref.transpose(2, 0, 1).reshape(D, BR))
    refC = swz_x(refT, BR)
    posC = swz_x(np.ascontiguousarray(pos.T), R)

    in_maps = []
    for c in range(NCORES):
        sl = slice(HPC * c, HPC * (c + 1))

        in_maps.append(
            {
                "refC": refC,
                "posC": posC,
                "wq": swz_w(wq[:, sl, :].reshape(D, HS)),
                "wk": swz_w(wk[:, sl, :].reshape(D, HS)),
                "wv": swz_w(wv[:, sl, :].reshape(D, HS)),
                "wr": swz_w(wr[:, sl, :].reshape(D, HS)),
                "wo": np.ascontiguousarray(
                    wo[sl, :, :].reshape(HS, D)
                ).astype(np.float16),
                "cb": np.ascontiguousarray(
                    cb[sl, :].reshape(HS, 1)
                ).astype(np.float32),
                "pb": np.ascontiguousarray(
                    pb[sl, :].reshape(HS, 1)
                ).astype(np.float32),
            }
        )
    return in_maps


def run(inputs, trace=False, **kw):
    global _CACHED_NC
    if _CACHED_NC is None:
        _CACHED_NC = build_nc()
    in_maps = _make_in_maps(inputs)
    res = run_bass_kernel_spmd(
        _CACHED_NC, in_maps, core_ids=list(range(NCORES)), trace=trace, **kw
    )
    y = np.zeros((BQ, D), dtype=np.float32)
    for r in res.results:
        y += r["out"].astype(np.float32)
    return y.reshape(B, Q, D), res


def kernel(**inputs):
    y, _ = run(inputs, trace=False)
    return y


# revision 46
# speedup vs baseline: 1.0085x; 1.0085x over previous
"""TransformerXL relative attention on 8 TRN2 NeuronCores.

Sharding: 16 heads -> 2 heads per core (tensor parallel). Each core computes
its column shard of the Q/K/V/R projections, full-batch attention for its two
heads, and the full-batch row-sharded output projection, producing a partial
[B*Q, D] output. The host sums the 8 partials (row-parallel matmul => the
all-reduce is a host-side sum).

v16 = v3 dataflow + fine-grained phase fusion:
  - The attention phase for batch b is EMISSION-INTERLEAVED with the
    projection/position phase of batch b+2 (and with the previous batch's
    output projection at the tail): after each content-matmul pair the
    emitter drops in one projection-chain unit (8 back-to-back matmuls).
    The engine queues are FIFO, so this keeps ~2us of independent PE work
    between every dependent attention step -- the PE no longer drains
    while DVE adds / ACT exps catch up, which both hides the softmax
    latency chain and keeps the HAM clock gate at full rate (isolated
    attention phases ran at the 1.2 GHz cold clock).
  - PE warmup matmuls during the initial loads (HAM ramp); posC/wr lead
    the scalar ring; the first ref chunks ride the idle sync ring.
  - each shifted XBAR-transposed st read is split into 2 chunks so the
    first adds start at half the transpose latency (4-way split loses:
    per-transpose fixed cost dominates); the ybuf write rides the sync
    ring directly ahead of its st chunks (no gpsimd->sync sem hop).
  - replicated-denominator attn@V: the lhsT is [128,128] = [V_h | 64
    ones columns], so o_ps rows 64:128 hold the softmax denominator
    replicated across partitions and the 1/sum epilogue is two
    same-queue DVE ops (reciprocal + multiply) with ZERO DMA hops. The
    old DRAM-broadcast chain crossed 4 engine queues at ~8-12us latency
    per head and its tiny reciprocal queued behind bulk filler work;
    this trick only pays off inside the fused schedule (standalone it
    lost to the [64,512] reciprocal cost).
  - V^T for both heads comes from ONE full-partition [128,1024] XBAR
    transpose per batch (64-partition sources transpose at ~half rate).
  - content/position biases folded into the Q-projection PSUM drain via
    ACT per-partition bias APs (removes the rank-1 matmuls and shortens
    the q chain); with the epilogue now DVE-resident, the projection
    and output drains all run on ACT to balance (DVE ~122us busy was
    2nd after PE; ACT had 45us of slack).
  - filler distribution balanced across ALL attention phases: pp3's
    units split between at1 and at2, and the output-projection phases
    folded in as fillers (schedule: pp0 pp1 [at0|pp2] [at1|pp3a+y0]
    [at2|pp3b+y1] [at3|y2] y3), so the tail attention phases keep
    independent PE work in every fill slot instead of running dry.

Measured (8-core SPMD, one traced run each): v3 baseline 232-270us
across runs (median ~259); this version 198-209us (median ~201, best
198.6). rel err 1.29e-3.
"""

import numpy as np

import concourse.bass as bass
import concourse.mybir as mybir
import concourse.tile as tile
from concourse import bacc
from concourse.bass_utils import run_bass_kernel_spmd

B, Q, M, D, H = 4, 512, 512, 1024, 16
S = D // H          # 64
R = Q + M           # 1024
NCORES = 8
HPC = H // NCORES   # heads per core = 2
HS = HPC * S        # per-core head-channel width = 128
BR = B * R          # 4096
BQ = B * Q          # 2048

FP16 = mybir.dt.float16
FP32 = mybir.dt.float32
AF = mybir.ActivationFunctionType

_CACHED_NC = None


def build_nc():
    nc = bacc.Bacc()

    refC = nc.declare_dram_parameter("refC", [BR // 512, 128, 8 * 512],
                                     FP16, isOutput=False)
    posC = nc.declare_dram_parameter("posC", [R // 512, 128, 8 * 512],
                                     FP16, isOutput=False)
    wq = nc.declare_dram_parameter("wq", [128, 8 * 128], FP16,
                                   isOutput=False)
    wk = nc.declare_dram_parameter("wk", [128, 8 * 128], FP16,
                                   isOutput=False)
    wv = nc.declare_dram_parameter("wv", [128, 8 * 128], FP16,
                                   isOutput=False)
    wr = nc.declare_dram_parameter("wr", [128, 8 * 128], FP16,
                                   isOutput=False)
    wo = nc.declare_dram_parameter("wo", [HS, D], FP16, isOutput=False)
    cb = nc.declare_dram_parameter("cb", [HS, 1], FP32, isOutput=False)
    pb = nc.declare_dram_parameter("pb", [HS, 1], FP32, isOutput=False)
    y_out = nc.declare_dram_parameter("out", [BQ, D], FP16, isOutput=True)

    DT = D // 128  # 8 contraction tiles
    KT = R // 128  # 8 key tiles per batch row-block

    with tile.TileContext(nc) as tc:
        with (
            tc.tile_pool(name="consts", bufs=1) as consts,
            tc.tile_pool(name="inputs", bufs=1) as inputs,
            tc.tile_pool(name="acts", bufs=1) as acts,
            tc.tile_pool(name="work", bufs=1) as work,
            tc.tile_pool(name="ps", bufs=1, space="PSUM") as ps,
            tc.tile_pool(name="dram", bufs=1, space="DRAM") as dram,
        ):
            # PE warmup: keep the HAM activity window busy while the first
            # parameter DMAs land, so real matmuls start at full clock.
            warm = consts.tile([128, 512], FP16, tag="warm")
            nc.vector.memset(warm, 0.125)
            for _ in range(24):
                ps_w = ps.tile([128, 512], FP32, tag="proj", bufs=2)
                nc.tensor.matmul(ps_w, warm[:, 0:128], warm,
                                 start=True, stop=True)

            def load_w(param, name):
                t = consts.tile([128, DT, 128], FP16, tag=name)
                nc.scalar.dma_start(
                    out=t, in_=param.rearrange("p (dt m) -> p dt m", dt=DT)
                )
                return t

            pos_cs = []
            ref_cs = []

            def load_chunk(view, c, lst, nm, queue=None):
                rc = inputs.tile([128, DT, 512], FP16, tag="ref", bufs=4,
                                 name=f"{nm}{c}")
                q = queue or nc.scalar
                q.dma_start(
                    out=rc,
                    in_=view[c].rearrange("p (dt j) -> p dt j", dt=DT),
                )
                lst.append(rc)

            load_chunk(posC, 0, pos_cs, "pos")
            wr_sb = load_w(wr, "wr")
            load_chunk(posC, 1, pos_cs, "pos")
            wk_sb = load_w(wk, "wk")
            wq_sb = load_w(wq, "wq")
            wv_sb = load_w(wv, "wv")
            load_chunk(refC, 0, ref_cs, "ref", queue=nc.sync)
            load_chunk(refC, 1, ref_cs, "ref", queue=nc.sync)
            wo_sb = consts.tile([HS, D], FP16, tag="wo")
            nc.scalar.dma_start(out=wo_sb, in_=wo[:, :])
            cb_sb = consts.tile([HS, 1], FP32, tag="cb")
            nc.scalar.dma_start(out=cb_sb, in_=cb[:, :])
            pb_sb = consts.tile([HS, 1], FP32, tag="pb")
            nc.scalar.dma_start(out=pb_sb, in_=pb[:, :])

            # persistent activations (all fp16)
            k_sbs = []
            qcb_sbs = []
            qpb_sbs = []
            o_sbs = []
            vt_bs = {}
            for bb in range(B):
                k_sbs.append(acts.tile([HS, R], FP16, tag=f"k{bb}",
                                       name=f"k{bb}"))
                qcb_sbs.append(acts.tile([HS, 512], FP16, tag=f"qcb{bb}",
                                         name=f"qcb{bb}"))
                qpb_sbs.append(acts.tile([HS, 512], FP16, tag=f"qpb{bb}",
                                         name=f"qpb{bb}"))
                o_sbs.append(acts.tile([HS, 512], FP16, tag=f"o{bb}",
                                       name=f"o{bb}"))
            rel_sb = acts.tile([HS, R], FP16, tag="rel_sb")
            v_sb = acts.tile([HS, BR], FP16, tag="v_sb")
            recbuf = dram.tile([B, 1024], FP16, tag="recbuf")
            st_alls = {}

            # rel projection (needed by every position phase)
            for c in range(R // 512):
                ps_r = ps.tile([128, 512], FP32, tag="proj", bufs=2)
                for dt in range(DT):
                    nc.tensor.matmul(
                        ps_r, wr_sb[:, dt, :], pos_cs[c][:, dt, :],
                        start=(dt == 0), stop=(dt == DT - 1),
                    )
                nc.scalar.activation(
                    rel_sb[:, c * 512:(c + 1) * 512], ps_r, AF.Copy
                )

            def emit_proj(wt, c, dst):
                """One projection chain: dst <- wt.T @ ref chunk c."""
                ps_t = ps.tile([128, 512], FP32, tag="proj", bufs=2)
                for dt in range(DT):
                    nc.tensor.matmul(
                        ps_t, wt[:, dt, :], ref_cs[c][:, dt, :],
                        start=(dt == 0), stop=(dt == DT - 1),
                    )
                nc.scalar.activation(dst, ps_t, AF.Copy)

            def emit_q(b):
                c = 2 * b + 1
                ps_q = ps.tile([128, 512], FP32, tag="proj", bufs=2)
                for dt in range(DT):
                    nc.tensor.matmul(
                        ps_q, wq_sb[:, dt, :], ref_cs[c][:, dt, :],
                        start=(dt == 0), stop=(dt == DT - 1),
                    )
                nc.scalar.activation(qcb_sbs[b], ps_q, AF.Identity,
                                     bias=cb_sb)
                nc.scalar.activation(qpb_sbs[b], ps_q, AF.Identity,
                                     bias=pb_sb)

            def emit_pos_mms(b, p_alls, qts):
                """Position-logit matmul pairs + PSUM drains for query
                tiles `qts` of batch b (heads paired in the PE array)."""
                for qt in qts:
                    for kh in range(2):
                        pss = []
                        for h in range(HPC):
                            hsl = slice(h * S, (h + 1) * S)
                            ps_p = ps.tile([128, 512], FP32,
                                           tag="pos", bufs=2, name="ps_p")
                            nc.tensor.matmul(
                                ps_p,
                                qpb_sbs[b][hsl, qt * 128:(qt + 1) * 128],
                                rel_sb[hsl, kh * 512:(kh + 1) * 512],
                                start=True, stop=True,
                                tile_position=(h * S, 0),
                            )
                            pss.append(ps_p)
                        for h in range(HPC):
                            dst = p_alls[h][:, qt, 1 + kh * 512:
                                            1 + (kh + 1) * 512]
                            if h == 0:
                                nc.scalar.activation(dst, pss[h], AF.Copy)
                            else:
                                nc.vector.tensor_copy(dst, pss[h])

            def emit_pos_rt(b, p_alls, h):
                """DRAM round trip for head h: contiguous ybuf write then
                the shifted XBAR-transposed read back."""
                ybuf = dram.tile(
                    [Q, R + 1], FP16, tag=f"ybuf{b}_{h}",
                    name=f"ybuf{b}_{h}",
                )
                nc.sync.dma_start(
                    out=ybuf.rearrange("(qt p) c -> p qt c", p=128),
                    in_=p_alls[h],
                )
                shifted = (
                    ybuf.rearrange("a b -> (a b)")[Q: Q + Q * R]
                    .rearrange("(q r) -> q r", r=R)
                )
                st_all = work.tile(
                    [128, KT, 512], FP16,
                    tag="st", name=f"st{b}_{h}", bufs=5,
                )
                for cc in range(2):
                    nc.sync.dma_start(
                        out=st_all[:, cc * 4:(cc + 1) * 4, :],
                        in_=shifted[:, cc * 512:(cc + 1) * 512],
                        transpose=True,
                    )
                st_alls[(b, h)] = st_all

            def emit_vt(b):
                """V^T (ones-augmented) for batch b: ONE full-partition
                [128,1024] XBAR transpose covers both heads (a 64-partition
                source transposes at ~half efficiency), then per-head
                ones-augmented lhsT tiles are sliced out on the DVE."""
                vt_both = work.tile(
                    [128, KT, 128], FP16,
                    tag="vt_both", name=f"vt_both{b}", bufs=2,
                )
                nc.sync.dma_start(
                    out=vt_both,
                    in_=v_sb[:, b * R:(b + 1) * R],
                    transpose=True,
                )
                for h in range(HPC):
                    vt_all = acts.tile(
                        [128, KT, 128], FP16,
                        tag=f"vt{b}_{h}", name=f"vt{b}_{h}",
                    )
                    nc.vector.memset(vt_all[:, :, S:], 1.0)
                    nc.vector.tensor_copy(
                        vt_all[:, :, 0:S],
                        vt_both[:, :, h * S:(h + 1) * S],
                    )
                    vt_bs[(b, h)] = vt_all

            def projpos_units(b):
                """emit_projpos(b) as a list of closures, each ~one
                projection chain of PE work, to interleave with attention."""
                p_alls = []
                for h in range(HPC):
                    p_all = work.tile(
                        [128, Q // 128, R + 1], FP16,
                        tag="p_all", name=f"p_all{b}_{h}", bufs=4,
                    )
                    nc.vector.memset(p_all[:, :, 0:1], 0.0)
                    p_alls.append(p_all)

                def u_k0():
                    if b < B - 1:
                        load_chunk(refC, 2 * b + 2, ref_cs, "ref")
                        load_chunk(refC, 2 * b + 3, ref_cs, "ref")
                    emit_proj(wk_sb, 2 * b, k_sbs[b][:, 0:512])

                return [
                    u_k0,
                    lambda: emit_proj(wv_sb, 2 * b,
                                      v_sb[:, b * R:b * R + 512]),
                    lambda: emit_q(b),
                    lambda: emit_pos_mms(b, p_alls, [0, 1]),
                    lambda: emit_pos_mms(b, p_alls, [2, 3]),
                    lambda: emit_pos_rt(b, p_alls, 0),
                    lambda: emit_proj(wk_sb, 2 * b + 1,
                                      k_sbs[b][:, 512:1024]),
                    lambda: emit_pos_rt(b, p_alls, 1),
                    lambda: emit_proj(wv_sb, 2 * b + 1,
                                      v_sb[:, b * R + 512:b * R + R]),
                    lambda: emit_vt(b),
                ]

            def outproj_units(b):
                def mk(t):
                    def u():
                        y_sb = work.tile([128, D], FP16, tag="y_sb", bufs=4)
                        for j in range(2):
                            y_ps = ps.tile([128, 512], FP32, tag="proj",
                                           bufs=2, name="y_ps")
                            nc.tensor.matmul(
                                y_ps,
                                o_sbs[b][:, t * 128:(t + 1) * 128],
                                wo_sb[:, j * 512:(j + 1) * 512],
                                start=True, stop=True,
                            )
                            nc.scalar.activation(
                                y_sb[:, j * 512:(j + 1) * 512], y_ps,
                                AF.Copy,
                            )
                        nc.scalar.dma_start(
                            out=y_out[(b * 4 + t) * 128:
                                      (b * 4 + t + 1) * 128, :],
                            in_=y_sb,
                        )
                    return u
                return [mk(t) for t in range(4)]

            def emit_attn(b, fillers):
                """Attention for batch b, with one filler unit (a chunk of
                independent PE work) emitted after each dependent step so
                the PE queue never drains while adds/exps catch up."""
                fillers = list(fillers)

                def fill():
                    if fillers:
                        fillers.pop(0)()

                exs = {h: [] for h in range(HPC)}
                for K in range(KT):
                    cts = []
                    for h in range(HPC):
                        hsl = slice(h * S, (h + 1) * S)
                        ct = ps.tile([128, 512], FP32, tag="ct",
                                     bufs=2, name=f"ct{h}")
                        nc.tensor.matmul(
                            ct,
                            k_sbs[b][hsl, K * 128:(K + 1) * 128],
                            qcb_sbs[b][hsl, :],
                            start=True, stop=True,
                            tile_position=(h * S, 0),
                        )
                        cts.append(ct)
                    l_sb = work.tile([128, 2, 512], FP16, tag="l_sb",
                                     name="l_sb", bufs=6)
                    for h in range(HPC):
                        nc.vector.tensor_add(
                            l_sb[:, h, :], cts[h], st_alls[(b, h)][:, K, :]
                        )
                    ex = work.tile([128, 2, 512], FP16, tag="ex",
                                   name="ex", bufs=10)
                    nc.scalar.activation(
                        ex, l_sb, AF.Exp, scale=1.0 / np.sqrt(S)
                    )
                    for h in range(HPC):
                        exs[h].append(ex[:, h, :])
                    fill()
                for h in range(HPC):
                    hsl = slice(h * S, (h + 1) * S)
                    o_ps = ps.tile([128, 512], FP32, tag="ov",
                                   bufs=2, name=f"o_ps{h}")
                    for K in range(KT):
                        nc.tensor.matmul(
                            o_ps,
                            vt_bs[(b, h)][:, K, :],
                            exs[h][K],
                            start=(K == 0), stop=(K == KT - 1),
                        )
                    # o_ps rows 64:128 hold the softmax denominator
                    # replicated across 64 partitions (the ones columns of
                    # vt), so the 1/sum epilogue is two same-queue DVE ops
                    # with zero DMA hops (the old DRAM-broadcast chain
                    # crossed 4 queues at ~8-12us latency)
                    rec_h = work.tile([S, 512], FP16, tag="rec",
                                      name=f"rec{h}", bufs=2)
                    with nc.allow_low_precision(reason="softmax 1/sum fp16"):
                        nc.vector.reciprocal(rec_h, o_ps[S:, :])
                    nc.vector.tensor_mul(
                        o_sbs[b][hsl, :],
                        o_ps[0:S, :],
                        rec_h,
                    )
                    fill()
                while fillers:
                    fillers.pop(0)()

            def emit_projpos(b):
                for u in projpos_units(b):
                    u()

            # schedule: pp0 pp1 [at0|pp2] [at1|pp3a+y0] [at2|pp3b+y1]
            # [at3|y2] y3 -- pp3's units split across at1/at2 and the y
            # phases folded in as fillers, so the tail attention phases
            # keep independent PE work in every fill slot.
            emit_projpos(0)
            emit_projpos(1)
            emit_attn(0, projpos_units(2))
            pp3 = projpos_units(3)
            y1u = outproj_units(1)
            emit_attn(1, pp3[0:6] + outproj_units(0))
            emit_attn(2, pp3[6:10] + y1u[0:2])
            emit_attn(3, y1u[2:4] + outproj_units(2))
            for u in outproj_units(3):
                u()

    nc.compile()
    return nc


def _make_in_maps(inputs):
    qs = np.asarray(inputs["query_seqs"], dtype=np.float32)
    pos = np.asarray(inputs["positional_encoding"], dtype=np.float32)
    mem = np.asarray(inputs["memory_seqs"], dtype=np.float32)
    wq = np.asarray(inputs["w_query"], dtype=np.float32)
    wk = np.asarray(inputs["w_key"], dtype=np.float32)
    wv = np.asarray(inputs["w_value"], dtype=np.float32)
    wr = np.asarray(inputs["w_r"], dtype=np.float32)
    wo = np.asarray(inputs["w_output"], dtype=np.float32)
    cb = np.asarray(inputs["content_bias"], dtype=np.float32)
    pb = np.asarray(inputs["position_bias"], dtype=np.float32)

    DT = D // 128

    def swz_w(w):
        # [D, HS] -> [128, DT*HS]: row p holds dt-major 128-blocks so the
        # SBUF load is per-partition contiguous.
        return np.ascontiguousarray(
            w.reshape(DT, 128, HS).transpose(1, 0, 2).reshape(128, DT * HS)
        ).astype(np.float16)

    def swz_x(xT, n):
        # [D, N] -> [N//512, 128, DT*512] chunk-major / partition / dt-major
        return np.ascontiguousarray(
            xT.reshape(DT, 128, n // 512, 512)
            .transpose(2, 1, 0, 3)
            .reshape(n // 512, 128, DT * 512)
        ).astype(np.float16)

    ref = np.concatenate([mem, qs], axis=1)  # [B, R, D]
    refT = np.ascontiguousarray(ref.transpose(2, 0, 1).reshape(D, BR))
    refC = swz_x(refT, BR)
    posC = swz_x(np.ascontiguousarray(pos.T), R)

    in_maps = []
    for c in range(NCORES):
        sl = slice(HPC * c, HPC * (c + 1))

        in_maps.append(
            {
                "refC": refC,
                "posC": posC,
                "wq": swz_w(wq[:, sl, :].reshape(D, HS)),
                "wk": swz_w(wk[:, sl, :].reshape(D, HS)),
                "wv": swz_w(wv[:, sl, :].reshape(D, HS)),
                "wr": swz_w(wr[:, sl, :].reshape(D, HS)),
                "wo": np.ascontiguousarray(
                    wo[sl, :, :].reshape(HS, D)
                ).astype(np.float16),
                "cb": np.ascontiguousarray(
                    cb[sl, :].reshape(HS, 1)
                ).astype(np.float32),
                "pb": np.ascontiguousarray(
                    pb[sl, :].reshape(HS, 1)
                ).astype(np.float32),
            }
        )
    return in_maps


def run(inputs, trace=False, **kw):
    global _CACHED_NC
    if _CACHED_NC is None:
        _CACHED_NC = build_nc()
    in_maps = _make_in_maps(inputs)
    res = run_bass_kernel_spmd(
        _CACHED_NC, in_maps, core_ids=list(range(NCORES)), trace=trace, **kw
    )
    y = np.zeros((BQ, D), dtype=np.float32)
    for r in res.results:
        y += r["out"].astype(np.float32)
    return y.reshape(B, Q, D), res


def kernel(**inputs):
    y, _ = run(inputs, trace=False)
    return y


# revision 47
# speedup vs baseline: 1.1087x; 1.0993x over previous
"""TransformerXL relative attention on 8 TRN2 NeuronCores.

Sharding: 16 heads -> 2 heads per core (tensor parallel). Each core computes
its column shard of the Q/K/V/R projections, full-batch attention for its two
heads, and the full-batch row-sharded output projection, producing a partial
[B*Q, D] output. The host sums the 8 partials (row-parallel matmul => the
all-reduce is a host-side sum).

v16 = v3 dataflow + fine-grained phase fusion:
  - The attention phase for batch b is EMISSION-INTERLEAVED with the
    projection/position phase of batch b+2 (and with the previous batch's
    output projection at the tail): after each content-matmul pair the
    emitter drops in one projection-chain unit (8 back-to-back matmuls).
    The engine queues are FIFO, so this keeps ~2us of independent PE work
    between every dependent attention step -- the PE no longer drains
    while DVE adds / ACT exps catch up, which both hides the softmax
    latency chain and keeps the HAM clock gate at full rate (isolated
    attention phases ran at the 1.2 GHz cold clock).
  - PE warmup matmuls during the initial loads (HAM ramp); posC/wr lead
    the scalar ring; the first ref chunks ride the idle sync ring.
  - each shifted XBAR-transposed st read is split into 2 chunks so the
    first adds start at half the transpose latency (4-way split loses:
    per-transpose fixed cost dominates); the ybuf write rides the sync
    ring directly ahead of its st chunks (no gpsimd->sync sem hop).
  - replicated-denominator attn@V: the lhsT is [128,128] = [V_h | 64
    ones columns], so o_ps rows 64:128 hold the softmax denominator
    replicated across partitions and the 1/sum epilogue is two
    same-queue DVE ops (reciprocal + multiply) with ZERO DMA hops. The
    old DRAM-broadcast chain crossed 4 engine queues at ~8-12us latency
    per head and its tiny reciprocal queued behind bulk filler work;
    this trick only pays off inside the fused schedule (standalone it
    lost to the [64,512] reciprocal cost).
  - V^T for both heads comes from ONE full-partition [128,1024] XBAR
    transpose per batch (64-partition sources transpose at ~half rate).
  - content/position biases folded into the Q-projection PSUM drain via
    ACT per-partition bias APs (removes the rank-1 matmuls and shortens
    the q chain); with the epilogue now DVE-resident, the projection
    and output drains all run on ACT to balance (DVE ~122us busy was
    2nd after PE; ACT had 45us of slack).
  - filler distribution balanced across ALL attention phases: pp3's
    units split between at1 and at2, the output-projection phases
    folded in as fillers, and y1's units split across at2/at3 so both
    tails get 6 units (schedule: pp0 pp1 [at0|pp2] [at1|pp3a+y0]
    [at2|pp3b+y1a] [at3|y1b+y2] y3).

Measured (8-core SPMD, one traced run each): v3 baseline 232-270us
across runs (median ~259); this version 199-214us across device states
(median ~204, best 198.6). rel err 1.29e-3.
"""

import numpy as np

import concourse.bass as bass
import concourse.mybir as mybir
import concourse.tile as tile
from concourse import bacc
from concourse.bass_utils import run_bass_kernel_spmd

B, Q, M, D, H = 4, 512, 512, 1024, 16
S = D // H          # 64
R = Q + M           # 1024
NCORES = 8
HPC = H // NCORES   # heads per core = 2
HS = HPC * S        # per-core head-channel width = 128
BR = B * R          # 4096
BQ = B * Q          # 2048

FP16 = mybir.dt.float16
FP32 = mybir.dt.float32
AF = mybir.ActivationFunctionType

_CACHED_NC = None


def build_nc():
    nc = bacc.Bacc()

    refC = nc.declare_dram_parameter("refC", [BR // 512, 128, 8 * 512],
                                     FP16, isOutput=False)
    posC = nc.declare_dram_parameter("posC", [R // 512, 128, 8 * 512],
                                     FP16, isOutput=False)
    wq = nc.declare_dram_parameter("wq", [128, 8 * 128], FP16,
                                   isOutput=False)
    wk = nc.declare_dram_parameter("wk", [128, 8 * 128], FP16,
                                   isOutput=False)
    wv = nc.declare_dram_parameter("wv", [128, 8 * 128], FP16,
                                   isOutput=False)
    wr = nc.declare_dram_parameter("wr", [128, 8 * 128], FP16,
                                   isOutput=False)
    wo = nc.declare_dram_parameter("wo", [HS, D], FP16, isOutput=False)
    cb = nc.declare_dram_parameter("cb", [HS, 1], FP32, isOutput=False)
    pb = nc.declare_dram_parameter("pb", [HS, 1], FP32, isOutput=False)
    y_out = nc.declare_dram_parameter("out", [BQ, D], FP16, isOutput=True)

    DT = D // 128  # 8 contraction tiles
    KT = R // 128  # 8 key tiles per batch row-block

    with tile.TileContext(nc) as tc:
        with (
            tc.tile_pool(name="consts", bufs=1) as consts,
            tc.tile_pool(name="inputs", bufs=1) as inputs,
            tc.tile_pool(name="acts", bufs=1) as acts,
            tc.tile_pool(name="work", bufs=1) as work,
            tc.tile_pool(name="ps", bufs=1, space="PSUM") as ps,
            tc.tile_pool(name="dram", bufs=1, space="DRAM") as dram,
        ):
            # PE warmup: keep the HAM activity window busy while the first
            # parameter DMAs land, so real matmuls start at full clock.
            warm = consts.tile([128, 512], FP16, tag="warm")
            nc.vector.memset(warm, 0.125)
            for _ in range(24):
                ps_w = ps.tile([128, 512], FP32, tag="proj", bufs=2)
                nc.tensor.matmul(ps_w, warm[:, 0:128], warm,
                                 start=True, stop=True)

            def load_w(param, name):
                t = consts.tile([128, DT, 128], FP16, tag=name)
                nc.scalar.dma_start(
                    out=t, in_=param.rearrange("p (dt m) -> p dt m", dt=DT)
                )
                return t

            pos_cs = []
            ref_cs = []

            def load_chunk(view, c, lst, nm, queue=None):
                rc = inputs.tile([128, DT, 512], FP16, tag="ref", bufs=4,
                                 name=f"{nm}{c}")
                q = queue or nc.scalar
                q.dma_start(
                    out=rc,
                    in_=view[c].rearrange("p (dt j) -> p dt j", dt=DT),
                )
                lst.append(rc)

            load_chunk(posC, 0, pos_cs, "pos")
            wr_sb = load_w(wr, "wr")
            load_chunk(posC, 1, pos_cs, "pos")
            wk_sb = load_w(wk, "wk")
            wq_sb = load_w(wq, "wq")
            wv_sb = load_w(wv, "wv")
            load_chunk(refC, 0, ref_cs, "ref", queue=nc.sync)
            load_chunk(refC, 1, ref_cs, "ref", queue=nc.sync)
            wo_sb = consts.tile([HS, D], FP16, tag="wo")
            nc.scalar.dma_start(out=wo_sb, in_=wo[:, :])
            cb_sb = consts.tile([HS, 1], FP32, tag="cb")
            nc.scalar.dma_start(out=cb_sb, in_=cb[:, :])
            pb_sb = consts.tile([HS, 1], FP32, tag="pb")
            nc.scalar.dma_start(out=pb_sb, in_=pb[:, :])

            # persistent activations (all fp16)
            k_sbs = []
            qcb_sbs = []
            qpb_sbs = []
            o_sbs = []
            vt_bs = {}
            for bb in range(B):
                k_sbs.append(acts.tile([HS, R], FP16, tag=f"k{bb}",
                                       name=f"k{bb}"))
                qcb_sbs.append(acts.tile([HS, 512], FP16, tag=f"qcb{bb}",
                                         name=f"qcb{bb}"))
                qpb_sbs.append(acts.tile([HS, 512], FP16, tag=f"qpb{bb}",
                                         name=f"qpb{bb}"))
                o_sbs.append(acts.tile([HS, 512], FP16, tag=f"o{bb}",
                                       name=f"o{bb}"))
            rel_sb = acts.tile([HS, R], FP16, tag="rel_sb")
            v_sb = acts.tile([HS, BR], FP16, tag="v_sb")
            recbuf = dram.tile([B, 1024], FP16, tag="recbuf")
            st_alls = {}

            # rel projection (needed by every position phase)
            for c in range(R // 512):
                ps_r = ps.tile([128, 512], FP32, tag="proj", bufs=2)
                for dt in range(DT):
                    nc.tensor.matmul(
                        ps_r, wr_sb[:, dt, :], pos_cs[c][:, dt, :],
                        start=(dt == 0), stop=(dt == DT - 1),
                    )
                nc.scalar.activation(
                    rel_sb[:, c * 512:(c + 1) * 512], ps_r, AF.Copy
                )

            def emit_proj(wt, c, dst):
                """One projection chain: dst <- wt.T @ ref chunk c."""
                ps_t = ps.tile([128, 512], FP32, tag="proj", bufs=2)
                for dt in range(DT):
                    nc.tensor.matmul(
                        ps_t, wt[:, dt, :], ref_cs[c][:, dt, :],
                        start=(dt == 0), stop=(dt == DT - 1),
                    )
                nc.scalar.activation(dst, ps_t, AF.Copy)

            def emit_q(b):
                c = 2 * b + 1
                ps_q = ps.tile([128, 512], FP32, tag="proj", bufs=2)
                for dt in range(DT):
                    nc.tensor.matmul(
                        ps_q, wq_sb[:, dt, :], ref_cs[c][:, dt, :],
                        start=(dt == 0), stop=(dt == DT - 1),
                    )
                nc.scalar.activation(qcb_sbs[b], ps_q, AF.Identity,
                                     bias=cb_sb)
                nc.scalar.activation(qpb_sbs[b], ps_q, AF.Identity,
                                     bias=pb_sb)

            def emit_pos_mms(b, p_alls, qts):
                """Position-logit matmul pairs + PSUM drains for query
                tiles `qts` of batch b (heads paired in the PE array)."""
                for qt in qts:
                    for kh in range(2):
                        pss = []
                        for h in range(HPC):
                            hsl = slice(h * S, (h + 1) * S)
                            ps_p = ps.tile([128, 512], FP32,
                                           tag="pos", bufs=2, name="ps_p")
                            nc.tensor.matmul(
                                ps_p,
                                qpb_sbs[b][hsl, qt * 128:(qt + 1) * 128],
                                rel_sb[hsl, kh * 512:(kh + 1) * 512],
                                start=True, stop=True,
                                tile_position=(h * S, 0),
                            )
                            pss.append(ps_p)
                        for h in range(HPC):
                            dst = p_alls[h][:, qt, 1 + kh * 512:
                                            1 + (kh + 1) * 512]
                            if h == 0:
                                nc.scalar.activation(dst, pss[h], AF.Copy)
                            else:
                                nc.vector.tensor_copy(dst, pss[h])

            def emit_pos_rt(b, p_alls, h):
                """DRAM round trip for head h: contiguous ybuf write then
                the shifted XBAR-transposed read back."""
                ybuf = dram.tile(
                    [Q, R + 1], FP16, tag=f"ybuf{b}_{h}",
                    name=f"ybuf{b}_{h}",
                )
                nc.sync.dma_start(
                    out=ybuf.rearrange("(qt p) c -> p qt c", p=128),
                    in_=p_alls[h],
                )
                shifted = (
                    ybuf.rearrange("a b -> (a b)")[Q: Q + Q * R]
                    .rearrange("(q r) -> q r", r=R)
                )
                st_all = work.tile(
                    [128, KT, 512], FP16,
                    tag="st", name=f"st{b}_{h}", bufs=5,
                )
                for cc in range(2):
                    nc.sync.dma_start(
                        out=st_all[:, cc * 4:(cc + 1) * 4, :],
                        in_=shifted[:, cc * 512:(cc + 1) * 512],
                        transpose=True,
                    )
                st_alls[(b, h)] = st_all

            def emit_vt(b):
                """V^T (ones-augmented) for batch b: ONE full-partition
                [128,1024] XBAR transpose covers both heads (a 64-partition
                source transposes at ~half efficiency), then per-head
                ones-augmented lhsT tiles are sliced out on the DVE."""
                vt_both = work.tile(
                    [128, KT, 128], FP16,
                    tag="vt_both", name=f"vt_both{b}", bufs=2,
                )
                nc.sync.dma_start(
                    out=vt_both,
                    in_=v_sb[:, b * R:(b + 1) * R],
                    transpose=True,
                )
                for h in range(HPC):
                    vt_all = acts.tile(
                        [128, KT, 128], FP16,
                        tag=f"vt{b}_{h}", name=f"vt{b}_{h}",
                    )
                    nc.vector.memset(vt_all[:, :, S:], 1.0)
                    nc.vector.tensor_copy(
                        vt_all[:, :, 0:S],
                        vt_both[:, :, h * S:(h + 1) * S],
                    )
                    vt_bs[(b, h)] = vt_all

            def projpos_units(b):
                """emit_projpos(b) as a list of closures, each ~one
                projection chain of PE work, to interleave with attention."""
                p_alls = []
                for h in range(HPC):
                    p_all = work.tile(
                        [128, Q // 128, R + 1], FP16,
                        tag="p_all", name=f"p_all{b}_{h}", bufs=4,
                    )
                    nc.vector.memset(p_all[:, :, 0:1], 0.0)
                    p_alls.append(p_all)

                def u_k0():
                    if b < B - 1:
                        load_chunk(refC, 2 * b + 2, ref_cs, "ref")
                        load_chunk(refC, 2 * b + 3, ref_cs, "ref")
                    emit_proj(wk_sb, 2 * b, k_sbs[b][:, 0:512])

                return [
                    u_k0,
                    lambda: emit_proj(wv_sb, 2 * b,
                                      v_sb[:, b * R:b * R + 512]),
                    lambda: emit_q(b),
                    lambda: emit_pos_mms(b, p_alls, [0, 1]),
                    lambda: emit_pos_mms(b, p_alls, [2, 3]),
                    lambda: emit_pos_rt(b, p_alls, 0),
                    lambda: emit_proj(wk_sb, 2 * b + 1,
                                      k_sbs[b][:, 512:1024]),
                    lambda: emit_pos_rt(b, p_alls, 1),
                    lambda: emit_proj(wv_sb, 2 * b + 1,
                                      v_sb[:, b * R + 512:b * R + R]),
                    lambda: emit_vt(b),
                ]

            def outproj_units(b):
                def mk(t):
                    def u():
                        y_sb = work.tile([128, D], FP16, tag="y_sb", bufs=4)
                        for j in range(2):
                            y_ps = ps.tile([128, 512], FP32, tag="proj",
                                           bufs=2, name="y_ps")
                            nc.tensor.matmul(
                                y_ps,
                                o_sbs[b][:, t * 128:(t + 1) * 128],
                                wo_sb[:, j * 512:(j + 1) * 512],
                                start=True, stop=True,
                            )
                            nc.scalar.activation(
                                y_sb[:, j * 512:(j + 1) * 512], y_ps,
                                AF.Copy,
                            )
                        nc.scalar.dma_start(
                            out=y_out[(b * 4 + t) * 128:
                                      (b * 4 + t + 1) * 128, :],
                            in_=y_sb,
                        )
                    return u
                return [mk(t) for t in range(4)]

            def emit_attn(b, fillers):
                """Attention for batch b, with one filler unit (a chunk of
                independent PE work) emitted after each dependent step so
                the PE queue never drains while adds/exps catch up."""
                fillers = list(fillers)

                def fill():
                    if fillers:
                        fillers.pop(0)()

                exs = {h: [] for h in range(HPC)}
                for K in range(KT):
                    cts = []
                    for h in range(HPC):
                        hsl = slice(h * S, (h + 1) * S)
                        ct = ps.tile([128, 512], FP32, tag="ct",
                                     bufs=2, name=f"ct{h}")
                        nc.tensor.matmul(
                            ct,
                            k_sbs[b][hsl, K * 128:(K + 1) * 128],
                            qcb_sbs[b][hsl, :],
                            start=True, stop=True,
                            tile_position=(h * S, 0),
                        )
                        cts.append(ct)
                    l_sb = work.tile([128, 2, 512], FP16, tag="l_sb",
                                     name="l_sb", bufs=6)
                    for h in range(HPC):
                        nc.vector.tensor_add(
                            l_sb[:, h, :], cts[h], st_alls[(b, h)][:, K, :]
                        )
                    ex = work.tile([128, 2, 512], FP16, tag="ex",
                                   name="ex", bufs=10)
                    nc.scalar.activation(
                        ex, l_sb, AF.Exp, scale=1.0 / np.sqrt(S)
                    )
                    for h in range(HPC):
                        exs[h].append(ex[:, h, :])
                    fill()
                for h in range(HPC):
                    hsl = slice(h * S, (h + 1) * S)
                    o_ps = ps.tile([128, 512], FP32, tag="ov",
                                   bufs=2, name=f"o_ps{h}")
                    for K in range(KT):
                        nc.tensor.matmul(
                            o_ps,
                            vt_bs[(b, h)][:, K, :],
                            exs[h][K],
                            start=(K == 0), stop=(K == KT - 1),
                        )
                    # o_ps rows 64:128 hold the softmax denominator
                    # replicated across 64 partitions (the ones columns of
                    # vt), so the 1/sum epilogue is two same-queue DVE ops
                    # with zero DMA hops (the old DRAM-broadcast chain
                    # crossed 4 queues at ~8-12us latency)
                    rec_h = work.tile([S, 512], FP16, tag="rec",
                                      name=f"rec{h}", bufs=2)
                    with nc.allow_low_precision(reason="softmax 1/sum fp16"):
                        nc.vector.reciprocal(rec_h, o_ps[S:, :])
                    nc.vector.tensor_mul(
                        o_sbs[b][hsl, :],
                        o_ps[0:S, :],
                        rec_h,
                    )
                    fill()
                while fillers:
                    fillers.pop(0)()

            def emit_projpos(b):
                for u in projpos_units(b):
                    u()

            # schedule: pp0 pp1 [at0|pp2] [at1|pp3a+y0] [at2|pp3b+y1]
            # [at3|y2] y3 -- pp3's units split across at1/at2 and the y
            # phases folded in as fillers, so the tail attention phases
            # keep independent PE work in every fill slot.
            emit_projpos(0)
            emit_projpos(1)
            emit_attn(0, projpos_units(2))
            pp3 = projpos_units(3)
            y1u = outproj_units(1)
            emit_attn(1, pp3[0:6] + outproj_units(0))
            emit_attn(2, pp3[6:10] + y1u[0:2])
            emit_attn(3, y1u[2:4] + outproj_units(2))
            for u in outproj_units(3):
                u()

    nc.compile()
    return nc


def _make_in_maps(inputs):
    qs = np.asarray(inputs["query_seqs"], dtype=np.float32)
    pos = np.asarray(inputs["positional_encoding"], dtype=np.float32)
    mem = np.asarray(inputs["memory_seqs"], dtype=np.float32)
    wq = np.asarray(inputs["w_query"], dtype=np.float32)
    wk = np.asarray(inputs["w_key"], dtype=np.float32)
    wv = np.asarray(inputs["w_value"], dtype=np.float32)
    wr = np.asarray(inputs["w_r"], dtype=np.float32)
    wo = np.asarray(inputs["w_output"], dtype=np.float32)
    cb = np.asarray(inputs["content_bias"], dtype=np.float32)
    pb = np.asarray(inputs["position_bias"], dtype=np.float32)

    DT = D // 128

    def swz_w(w):
        # [D, HS] -> [128, DT*HS]: row p holds dt-major 128-blocks so the
        # SBUF load is per-partition contiguous.
        return np.ascontiguousarray(
            w.reshape(DT, 128, HS).transpose(1, 0, 2).reshape(128, DT * HS)
        ).astype(np.float16)

    def swz_x(xT, n):
        # [D, N] -> [N//512, 128, DT*512] chunk-major / partition / dt-major
        return np.ascontiguousarray(
            xT.reshape(DT, 128, n // 512, 512)
            .transpose(2, 1, 0, 3)
            .reshape(n // 512, 128, DT * 512)
        ).astype(np.float16)

    ref = np.concatenate([mem, qs], axis=1)  # [B, R, D]
    refT = np.ascontiguousarray(ref.transpose(2, 0, 1).reshape(D, BR))
    refC = swz_x(refT, BR)
    posC = swz_x(np.ascontiguousarray(pos.T), R)

    in_maps = []
    for c in range(NCORES):
        sl = slice(HPC * c, HPC * (c + 1))

        in_maps.append(
            {
                "refC": refC,
                "posC": posC,
                "wq": swz_w(wq[:, sl, :].reshape(D, HS)),
                "wk": swz_w(wk[:, sl, :].reshape(D, HS)),
                "wv": swz_w(wv[:, sl, :].reshape(D, HS)),
                "wr": swz_w(wr[:, sl, :].reshape(D, HS)),
                "wo": np.ascontiguousarray(
                    wo[sl, :, :].reshape(HS, D)
                ).astype(np.float16),
                "cb": np.ascontiguousarray(
                    cb[sl, :].reshape(HS, 1)
                ).astype(np.float32),
                "pb": np.ascontiguousarray(
                    pb[sl, :].reshape(HS, 1)
                ).astype(np.float32),
            }
        )
    return in_maps


def run(inputs, trace=False, **kw):
    global _CACHED_NC
    if _CACHED_NC is None:
        _CACHED_NC = build_nc()
    in_maps = _make_in_maps(inputs)
    res = run_bass_kernel_spmd(
        _CACHED_NC, in_maps, core_ids=list(range(NCORES)), trace=trace, **kw
    )
    y = np.zeros((BQ, D), dtype=np.float32)
    for r in res.results:
        y += r["out"].astype(np.float32)
    return y.reshape(B, Q, D), res


def kernel(**inputs):
    y, _ = run(inputs, trace=False)
    return y


# revision 48
# speedup vs baseline: 1.1126x; 1.0035x over previous
"""TransformerXL relative attention on 8 TRN2 NeuronCores.

Sharding: 16 heads -> 2 heads per core (tensor parallel). Each core computes
its column shard of the Q/K/V/R projections, full-batch attention for its two
heads, and the full-batch row-sharded output projection, producing a partial
[B*Q, D] output. The host sums the 8 partials (row-parallel matmul => the
all-reduce is a host-side sum).

v16 = v3 dataflow + fine-grained phase fusion:
  - The attention phase for batch b is EMISSION-INTERLEAVED with the
    projection/position phase of batch b+2 (and with the previous batch's
    output projection at the tail): after each content-matmul pair the
    emitter drops in one projection-chain unit (8 back-to-back matmuls).
    The engine queues are FIFO, so this keeps ~2us of independent PE work
    between every dependent attention step -- the PE no longer drains
    while DVE adds / ACT exps catch up, which both hides the softmax
    latency chain and keeps the HAM clock gate at full rate (isolated
    attention phases ran at the 1.2 GHz cold clock).
  - PE warmup matmuls during the initial loads (HAM ramp); posC/wr lead
    the scalar ring; the first ref chunks ride the idle sync ring.
  - each shifted XBAR-transposed st read is split into 2 chunks so the
    first adds start at half the transpose latency (4-way split loses:
    per-transpose fixed cost dominates); the ybuf write rides the sync
    ring directly ahead of its st chunks (no gpsimd->sync sem hop).
  - replicated-denominator attn@V: the lhsT is [128,128] = [V_h | 64
    ones columns], so o_ps rows 64:128 hold the softmax denominator
    replicated across partitions and the 1/sum epilogue is two
    same-queue DVE ops (reciprocal + multiply) with ZERO DMA hops. The
    old DRAM-broadcast chain crossed 4 engine queues at ~8-12us latency
    per head and its tiny reciprocal queued behind bulk filler work;
    this trick only pays off inside the fused schedule (standalone it
    lost to the [64,512] reciprocal cost).
  - V^T for both heads comes from ONE full-partition [128,1024] XBAR
    transpose per batch (64-partition sources transpose at ~half rate).
  - content/position biases folded into the Q-projection PSUM drain via
    ACT per-partition bias APs (removes the rank-1 matmuls and shortens
    the q chain); with the epilogue now DVE-resident, the projection
    and output drains all run on ACT to balance (DVE ~122us busy was
    2nd after PE; ACT had 45us of slack).
  - filler distribution balanced across ALL attention phases: pp3's
    units split between at1 and at2, the output-projection phases
    folded in as fillers, and y1's units split across at2/at3 so both
    tails get 6 units (schedule: pp0 pp1 [at0|pp2] [at1|pp3a+y0]
    [at2|pp3b+y1a] [at3|y1b+y2] y3).

Measured (8-core SPMD, one traced run each): v3 baseline 232-270us
across runs (median ~259); this version 193-213us across device states
(median ~201, best 193.3). rel err 1.29e-3.
"""

import numpy as np

import concourse.bass as bass
import concourse.mybir as mybir
import concourse.tile as tile
from concourse import bacc
from concourse.bass_utils import run_bass_kernel_spmd

B, Q, M, D, H = 4, 512, 512, 1024, 16
S = D // H          # 64
R = Q + M           # 1024
NCORES = 8
HPC = H // NCORES   # heads per core = 2
HS = HPC * S        # per-core head-channel width = 128
BR = B * R          # 4096
BQ = B * Q          # 2048

FP16 = mybir.dt.float16
FP32 = mybir.dt.float32
AF = mybir.ActivationFunctionType

_CACHED_NC = None


def build_nc():
    nc = bacc.Bacc()

    refC = nc.declare_dram_parameter("refC", [BR // 512, 128, 8 * 512],
                                     FP16, isOutput=False)
    posC = nc.declare_dram_parameter("posC", [R // 512, 128, 8 * 512],
                                     FP16, isOutput=False)
    wq = nc.declare_dram_parameter("wq", [128, 8 * 128], FP16,
                                   isOutput=False)
    wk = nc.declare_dram_parameter("wk", [128, 8 * 128], FP16,
                                   isOutput=False)
    wv = nc.declare_dram_parameter("wv", [128, 8 * 128], FP16,
                                   isOutput=False)
    wr = nc.declare_dram_parameter("wr", [128, 8 * 128], FP16,
                                   isOutput=False)
    wo = nc.declare_dram_parameter("wo", [HS, D], FP16, isOutput=False)
    cb = nc.declare_dram_parameter("cb", [HS, 1], FP32, isOutput=False)
    pb = nc.declare_dram_parameter("pb", [HS, 1], FP32, isOutput=False)
    y_out = nc.declare_dram_parameter("out", [BQ, D], FP16, isOutput=True)

    DT = D // 128  # 8 contraction tiles
    KT = R // 128  # 8 key tiles per batch row-block

    with tile.TileContext(nc) as tc:
        with (
            tc.tile_pool(name="consts", bufs=1) as consts,
            tc.tile_pool(name="inputs", bufs=1) as inputs,
            tc.tile_pool(name="acts", bufs=1) as acts,
            tc.tile_pool(name="work", bufs=1) as work,
            tc.tile_pool(name="ps", bufs=1, space="PSUM") as ps,
            tc.tile_pool(name="dram", bufs=1, space="DRAM") as dram,
        ):
            # PE warmup: keep the HAM activity window busy while the first
            # parameter DMAs land, so real matmuls start at full clock.
            warm = consts.tile([128, 512], FP16, tag="warm")
            nc.vector.memset(warm, 0.125)
            for _ in range(24):
                ps_w = ps.tile([128, 512], FP32, tag="proj", bufs=2)
                nc.tensor.matmul(ps_w, warm[:, 0:128], warm,
                                 start=True, stop=True)

            def load_w(param, name):
                t = consts.tile([128, DT, 128], FP16, tag=name)
                nc.scalar.dma_start(
                    out=t, in_=param.rearrange("p (dt m) -> p dt m", dt=DT)
                )
                return t

            pos_cs = []
            ref_cs = []

            def load_chunk(view, c, lst, nm, queue=None):
                rc = inputs.tile([128, DT, 512], FP16, tag="ref", bufs=4,
                                 name=f"{nm}{c}")
                q = queue or nc.scalar
                q.dma_start(
                    out=rc,
                    in_=view[c].rearrange("p (dt j) -> p dt j", dt=DT),
                )
                lst.append(rc)

            load_chunk(posC, 0, pos_cs, "pos")
            wr_sb = load_w(wr, "wr")
            load_chunk(posC, 1, pos_cs, "pos")
            wk_sb = load_w(wk, "wk")
            wq_sb = load_w(wq, "wq")
            wv_sb = load_w(wv, "wv")
            load_chunk(refC, 0, ref_cs, "ref", queue=nc.sync)
            load_chunk(refC, 1, ref_cs, "ref", queue=nc.sync)
            wo_sb = consts.tile([HS, D], FP16, tag="wo")
            nc.scalar.dma_start(out=wo_sb, in_=wo[:, :])
            cb_sb = consts.tile([HS, 1], FP32, tag="cb")
            nc.scalar.dma_start(out=cb_sb, in_=cb[:, :])
            pb_sb = consts.tile([HS, 1], FP32, tag="pb")
            nc.scalar.dma_start(out=pb_sb, in_=pb[:, :])

            # persistent activations (all fp16)
            k_sbs = []
            qcb_sbs = []
            qpb_sbs = []
            o_sbs = []
            vt_bs = {}
            for bb in range(B):
                k_sbs.append(acts.tile([HS, R], FP16, tag=f"k{bb}",
                                       name=f"k{bb}"))
                qcb_sbs.append(acts.tile([HS, 512], FP16, tag=f"qcb{bb}",
                                         name=f"qcb{bb}"))
                qpb_sbs.append(acts.tile([HS, 512], FP16, tag=f"qpb{bb}",
                                         name=f"qpb{bb}"))
                o_sbs.append(acts.tile([HS, 512], FP16, tag=f"o{bb}",
                                       name=f"o{bb}"))
            rel_sb = acts.tile([HS, R], FP16, tag="rel_sb")
            v_sb = acts.tile([HS, BR], FP16, tag="v_sb")
            recbuf = dram.tile([B, 1024], FP16, tag="recbuf")
            st_alls = {}

            # rel projection (needed by every position phase)
            for c in range(R // 512):
                ps_r = ps.tile([128, 512], FP32, tag="proj", bufs=2)
                for dt in range(DT):
                    nc.tensor.matmul(
                        ps_r, wr_sb[:, dt, :], pos_cs[c][:, dt, :],
                        start=(dt == 0), stop=(dt == DT - 1),
                    )
                nc.scalar.activation(
                    rel_sb[:, c * 512:(c + 1) * 512], ps_r, AF.Copy
                )

            def emit_proj(wt, c, dst):
                """One projection chain: dst <- wt.T @ ref chunk c."""
                ps_t = ps.tile([128, 512], FP32, tag="proj", bufs=2)
                for dt in range(DT):
                    nc.tensor.matmul(
                        ps_t, wt[:, dt, :], ref_cs[c][:, dt, :],
                        start=(dt == 0), stop=(dt == DT - 1),
                    )
                nc.scalar.activation(dst, ps_t, AF.Copy)

            def emit_q(b):
                c = 2 * b + 1
                ps_q = ps.tile([128, 512], FP32, tag="proj", bufs=2)
                for dt in range(DT):
                    nc.tensor.matmul(
                        ps_q, wq_sb[:, dt, :], ref_cs[c][:, dt, :],
                        start=(dt == 0), stop=(dt == DT - 1),
                    )
                nc.scalar.activation(qcb_sbs[b], ps_q, AF.Identity,
                                     bias=cb_sb)
                nc.scalar.activation(qpb_sbs[b], ps_q, AF.Identity,
                                     bias=pb_sb)

            def emit_pos_mms(b, p_alls, qts):
                """Position-logit matmul pairs + PSUM drains for query
                tiles `qts` of batch b (heads paired in the PE array)."""
                for qt in qts:
                    for kh in range(2):
                        pss = []
                        for h in range(HPC):
                            hsl = slice(h * S, (h + 1) * S)
                            ps_p = ps.tile([128, 512], FP32,
                                           tag="pos", bufs=2, name="ps_p")
                            nc.tensor.matmul(
                                ps_p,
                                qpb_sbs[b][hsl, qt * 128:(qt + 1) * 128],
                                rel_sb[hsl, kh * 512:(kh + 1) * 512],
                                start=True, stop=True,
                                tile_position=(h * S, 0),
                            )
                            pss.append(ps_p)
                        for h in range(HPC):
                            dst = p_alls[h][:, qt, 1 + kh * 512:
                                            1 + (kh + 1) * 512]
                            if h == 0:
                                nc.scalar.activation(dst, pss[h], AF.Copy)
                            else:
                                nc.vector.tensor_copy(dst, pss[h])

            def emit_pos_rt(b, p_alls, h):
                """DRAM round trip for head h: contiguous ybuf write then
                the shifted XBAR-transposed read back."""
                ybuf = dram.tile(
                    [Q, R + 1], FP16, tag=f"ybuf{b}_{h}",
                    name=f"ybuf{b}_{h}",
                )
                nc.sync.dma_start(
                    out=ybuf.rearrange("(qt p) c -> p qt c", p=128),
                    in_=p_alls[h],
                )
                shifted = (
                    ybuf.rearrange("a b -> (a b)")[Q: Q + Q * R]
                    .rearrange("(q r) -> q r", r=R)
                )
                st_all = work.tile(
                    [128, KT, 512], FP16,
                    tag="st", name=f"st{b}_{h}", bufs=5,
                )
                for cc in range(2):
                    nc.sync.dma_start(
                        out=st_all[:, cc * 4:(cc + 1) * 4, :],
                        in_=shifted[:, cc * 512:(cc + 1) * 512],
                        transpose=True,
                    )
                st_alls[(b, h)] = st_all

            def emit_vt(b):
                """V^T (ones-augmented) for batch b: ONE full-partition
                [128,1024] XBAR transpose covers both heads (a 64-partition
                source transposes at ~half efficiency), then per-head
                ones-augmented lhsT tiles are sliced out on the DVE."""
                vt_both = work.tile(
                    [128, KT, 128], FP16,
                    tag="vt_both", name=f"vt_both{b}", bufs=2,
                )
                nc.sync.dma_start(
                    out=vt_both,
                    in_=v_sb[:, b * R:(b + 1) * R],
                    transpose=True,
                )
                for h in range(HPC):
                    vt_all = acts.tile(
                        [128, KT, 128], FP16,
                        tag=f"vt{b}_{h}", name=f"vt{b}_{h}",
                    )
                    nc.vector.memset(vt_all[:, :, S:], 1.0)
                    nc.vector.tensor_copy(
                        vt_all[:, :, 0:S],
                        vt_both[:, :, h * S:(h + 1) * S],
                    )
                    vt_bs[(b, h)] = vt_all

            def projpos_units(b):
                """emit_projpos(b) as a list of closures, each ~one
                projection chain of PE work, to interleave with attention."""
                p_alls = []
                for h in range(HPC):
                    p_all = work.tile(
                        [128, Q // 128, R + 1], FP16,
                        tag="p_all", name=f"p_all{b}_{h}", bufs=4,
                    )
                    nc.vector.memset(p_all[:, :, 0:1], 0.0)
                    p_alls.append(p_all)

                def u_k0():
                    if b < B - 1:
                        load_chunk(refC, 2 * b + 2, ref_cs, "ref")
                        load_chunk(refC, 2 * b + 3, ref_cs, "ref")
                    emit_proj(wk_sb, 2 * b, k_sbs[b][:, 0:512])

                return [
                    u_k0,
                    lambda: emit_proj(wv_sb, 2 * b,
                                      v_sb[:, b * R:b * R + 512]),
                    lambda: emit_q(b),
                    lambda: emit_pos_mms(b, p_alls, [0, 1]),
                    lambda: emit_pos_mms(b, p_alls, [2, 3]),
                    lambda: emit_pos_rt(b, p_alls, 0),
                    lambda: emit_proj(wk_sb, 2 * b + 1,
                                      k_sbs[b][:, 512:1024]),
                    lambda: emit_pos_rt(b, p_alls, 1),
                    lambda: emit_proj(wv_sb, 2 * b + 1,
                                      v_sb[:, b * R + 512:b * R + R]),
                    lambda: emit_vt(b),
                ]

            def outproj_units(b):
                def mk(t):
                    def u():
                        y_sb = work.tile([128, D], FP16, tag="y_sb", bufs=4)
                        for j in range(2):
                            y_ps = ps.tile([128, 512], FP32, tag="proj",
                                           bufs=2, name="y_ps")
                            nc.tensor.matmul(
                                y_ps,
                                o_sbs[b][:, t * 128:(t + 1) * 128],
                                wo_sb[:, j * 512:(j + 1) * 512],
                                start=True, stop=True,
                            )
                            nc.scalar.activation(
                                y_sb[:, j * 512:(j + 1) * 512], y_ps,
                                AF.Copy,
                            )
                        nc.scalar.dma_start(
                            out=y_out[(b * 4 + t) * 128:
                                      (b * 4 + t + 1) * 128, :],
                            in_=y_sb,
                        )
                    return u
                return [mk(t) for t in range(4)]

            def emit_attn(b, fillers):
                """Attention for batch b, with one filler unit (a chunk of
                independent PE work) emitted after each dependent step so
                the PE queue never drains while adds/exps catch up."""
                fillers = list(fillers)

                def fill():
                    if fillers:
                        fillers.pop(0)()

                exs = {h: [] for h in range(HPC)}
                for K in range(KT):
                    cts = []
                    for h in range(HPC):
                        hsl = slice(h * S, (h + 1) * S)
                        ct = ps.tile([128, 512], FP32, tag="ct",
                                     bufs=2, name=f"ct{h}")
                        nc.tensor.matmul(
                            ct,
                            k_sbs[b][hsl, K * 128:(K + 1) * 128],
                            qcb_sbs[b][hsl, :],
                            start=True, stop=True,
                            tile_position=(h * S, 0),
                        )
                        cts.append(ct)
                    l_sb = work.tile([128, 2, 512], FP16, tag="l_sb",
                                     name="l_sb", bufs=6)
                    for h in range(HPC):
                        nc.vector.tensor_add(
                            l_sb[:, h, :], cts[h], st_alls[(b, h)][:, K, :]
                        )
                    ex = work.tile([128, 2, 512], FP16, tag="ex",
                                   name="ex", bufs=10)
                    nc.scalar.activation(
                        ex, l_sb, AF.Exp, scale=1.0 / np.sqrt(S)
                    )
                    for h in range(HPC):
                        exs[h].append(ex[:, h, :])
                    fill()
                for h in range(HPC):
                    hsl = slice(h * S, (h + 1) * S)
                    o_ps = ps.tile([128, 512], FP32, tag="ov",
                                   bufs=2, name=f"o_ps{h}")
                    for K in range(KT):
                        nc.tensor.matmul(
                            o_ps,
                            vt_bs[(b, h)][:, K, :],
                            exs[h][K],
                            start=(K == 0), stop=(K == KT - 1),
                        )
                    # o_ps rows 64:128 hold the softmax denominator
                    # replicated across 64 partitions (the ones columns of
                    # vt), so the 1/sum epilogue is two same-queue DVE ops
                    # with zero DMA hops (the old DRAM-broadcast chain
                    # crossed 4 queues at ~8-12us latency)
                    rec_h = work.tile([S, 512], FP16, tag="rec",
                                      name=f"rec{h}", bufs=2)
                    with nc.allow_low_precision(reason="softmax 1/sum fp16"):
                        nc.vector.reciprocal(rec_h, o_ps[S:, :])
                    nc.vector.tensor_mul(
                        o_sbs[b][hsl, :],
                        o_ps[0:S, :],
                        rec_h,
                    )
                    fill()
                while fillers:
                    fillers.pop(0)()

            def emit_projpos(b):
                for u in projpos_units(b):
                    u()

            # schedule: pp0 pp1 [at0|pp2] [at1|pp3a+y0] [at2|pp3b+y1]
            # [at3|y2] y3 -- pp3's units split across at1/at2 and the y
            # phases folded in as fillers, so the tail attention phases
            # keep independent PE work in every fill slot.
            emit_projpos(0)
            emit_projpos(1)
            emit_attn(0, projpos_units(2))
            pp3 = projpos_units(3)
            y1u = outproj_units(1)
            emit_attn(1, pp3[0:6] + outproj_units(0))
            emit_attn(2, pp3[6:10] + y1u[0:2])
            emit_attn(3, y1u[2:4] + outproj_units(2))
            for u in outproj_units(3):
                u()

    nc.compile()
    return nc


def _make_in_maps(inputs):
    qs = np.asarray(inputs["query_seqs"], dtype=np.float32)
    pos = np.asarray(inputs["positional_encoding"], dtype=np.float32)
    mem = np.asarray(inputs["memory_seqs"], dtype=np.float32)
    wq = np.asarray(inputs["w_query"], dtype=np.float32)
    wk = np.asarray(inputs["w_key"], dtype=np.float32)
    wv = np.asarray(inputs["w_value"], dtype=np.float32)
    wr = np.asarray(inputs["w_r"], dtype=np.float32)
    wo = np.asarray(inputs["w_output"], dtype=np.float32)
    cb = np.asarray(inputs["content_bias"], dtype=np.float32)
    pb = np.asarray(inputs["position_bias"], dtype=np.float32)

    DT = D // 128

    def swz_w(w):
        # [D, HS] -> [128, DT*HS]: row p holds dt-major 128-blocks so the
        # SBUF load is per-partition contiguous.
        return np.ascontiguousarray(
            w.reshape(DT, 128, HS).transpose(1, 0, 2).reshape(128, DT * HS)
        ).astype(np.float16)

    def swz_x(xT, n):
        # [D, N] -> [N//512, 128, DT*512] chunk-major / partition / dt-major
        return np.ascontiguousarray(
            xT.reshape(DT, 128, n // 512, 512)
            .transpose(2, 1, 0, 3)
            .reshape(n // 512, 128, DT * 512)
        ).astype(np.float16)

    ref = np.concatenate([mem, qs], axis=1)  # [B, R, D]
    refT = np.ascontiguousarray(ref.transpose(2, 0, 1).reshape(D, BR))
    refC = swz_x(refT, BR)
    posC = swz_x(np.ascontiguousarray(pos.T), R)

    in_maps = []
    for c in range(NCORES):
        sl = slice(HPC * c, HPC * (c + 1))

        in_maps.append(
            {
                "refC": refC,
                "posC": posC,
                "wq": swz_w(wq[:, sl, :].reshape(D, HS)),
                "wk": swz_w(wk[:, sl, :].reshape(D, HS)),
                "wv": swz_w(wv[:, sl, :].reshape(D, HS)),
                "wr": swz_w(wr[:, sl, :].reshape(D, HS)),
                "wo": np.ascontiguousarray(
                    wo[sl, :, :].reshape(HS, D)
                ).astype(np.float16),
                "cb": np.ascontiguousarray(
                    cb[sl, :].reshape(HS, 1)
                ).astype(np.float32),
                "pb": np.ascontiguousarray(
                    pb[sl, :].reshape(HS, 1)
                ).astype(np.float32),
            }
        )
    return in_maps


def run(inputs, trace=False, **kw):
    global _CACHED_NC
    if _CACHED_NC is None:
        _CACHED_NC = build_nc()
    in_maps = _make_in_maps(inputs)
    res = run_bass_kernel_spmd(
        _CACHED_NC, in_maps, core_ids=list(range(NCORES)), trace=trace, **kw
    )
    y = np.zeros((BQ, D), dtype=np.float32)
    for r in res.results:
        y += r["out"].astype(np.float32)
    return y.reshape(B, Q, D), res


def kernel(**inputs):
    y, _ = run(inputs, trace=False)
    return y
